# revision 1
# baseline (speedup 1.0000x reference)
"""Trainium2 Bass kernel for nn_LowRankRotatedSpaceIntervention.

Reference computation (B=8192, D=4096, r=512, k=128):
    sel  = subspaces[0]                  # shared index set (fast path)
    diff = (source - base) @ W           # [B, r]
    out  = base + diff[:, sel] @ W[:, sel].T

Only the selected k=128 columns of W matter:
    out = base + ((source - base) @ W_sel) @ W_sel.T,  W_sel = W[:, sel]

Sharding: data-parallel over batch across 8 NeuronCores; W_sel (2 MiB)
replicated. Host precomputes W_sel and W_sel.T (cheap) from subspaces[0].

Device kernel per core (batch shard 1024 rows, 8 blocks of 128):
    load base/source block [128, 4096] f32
    D  = source - base                    (DVE, output bf16)
    Dt = PE-transpose of D in [128,128] chunks (bf16, via identity matmul)
    T^T[k,128] = sum_j W_sel_chunk_j.T @ Dt_chunk_j   (32 bf16 matmuls, psum)
    out_block = base + (T^T).T @ W_selT   (8 fp32 matmuls N=512 + DVE add)
    store out_block

The correction term has rms ~0.25 vs base ~1.0, so bf16 rounding on the
first matmul contributes ~1e-3 absolute error on the output; the second
matmul and the final add are fp32.
"""

import os
import numpy as np
import ml_dtypes

import concourse.bass as bass
import concourse.tile as tile
from concourse import bacc, masks, mybir
from concourse.bass_utils import run_bass_kernel_spmd

N_CORES = 8
B_FULL = 8192
D = 4096
K = 128
BS = B_FULL // N_CORES  # 1024 rows per core
NB = BS // 128          # 8 blocks of 128 rows
NCH = D // 128          # 32 contraction chunks of 128

F32 = mybir.dt.float32
BF16 = mybir.dt.bfloat16


UNIT_LAYOUTS = {
    "pairs": [(0, 1), (2, 3), (4, 5), (6, 7)],
    "tail_singles": [(0, 1), (2, 3), (4, 5), (6,), (7,)],
    "singles": [(i,) for i in range(8)],
    # same unit shape, but tail singles transpose via DMA xbar instead of PE
    "tail_dma": [(0, 1), (2, 3), (4, 5), (6,), (7,)],
    "all_dma": [(0, 1), (2, 3), (4, 5), (6,), (7,)],
    # singles at both ends: fast pipeline ramp-up AND short tail chain
    "ends_singles": [(0,), (1,), (2, 3), (4, 5), (6,), (7,)],
}


def _build(mm1_dtype="bf16", mm2_f32r=False, layout="tail_singles", deep_bufs=False):
    nc = bacc.Bacc("TRN2", target_bir_lowering=False, debug=False)

    base_d = nc.dram_tensor("base", [BS, D], F32, kind="ExternalInput").ap()
    src_d = nc.dram_tensor("source", [BS, D], F32, kind="ExternalInput").ap()
    w1_dt = BF16 if mm1_dtype == "bf16" else F32
    # fp32r is bit-identical to fp32; declaring the whole w2/ttt path as
    # fp32r satisfies the BIR verifier's "rounded to FP32r" producer rule.
    w2_dt = mybir.dt.float32r if mm2_f32r else F32
    # w1: chunk-major W_sel: w1[p, 128*j + k] = W_sel[128*j + p, k]
    w1_d = nc.dram_tensor("w1", [128, D], w1_dt, kind="ExternalInput").ap()
    # w2: W_sel.T  (k on partitions)
    w2_d = nc.dram_tensor("w2", [K, D], w2_dt, kind="ExternalInput").ap()
    out_d = nc.dram_tensor("out", [BS, D], F32, kind="ExternalOutput").ap()

    with tile.TileContext(nc) as tc:
        with (
            tc.tile_pool(name="wpool", bufs=1) as wpool,
            tc.tile_pool(name="ipool", bufs=1) as ipool,
            tc.tile_pool(name="spool", bufs=4 if deep_bufs else 3) as spool,
            tc.tile_pool(name="dpool", bufs=2) as dpool,
            tc.tile_pool(name="dtpool", bufs=2) as dtpool,
            tc.tile_pool(name="ttpool", bufs=2) as ttpool,
            tc.tile_pool(name="opool", bufs=5 if deep_bufs else 4) as opool,
            tc.tile_pool(name="ptr", bufs=2, space="PSUM") as ptrpool,
            tc.tile_pool(name="pT", bufs=2, space="PSUM") as pTpool,
            tc.tile_pool(name="p2", bufs=4, space="PSUM") as p2pool,
        ):
            w1_sb = wpool.tile([128, D], w1_dt, tag="w1")
            nc.sync.dma_start(w1_sb[:], w1_d[:])
            w2_sb = wpool.tile([K, D], w2_dt, tag="w2")
            nc.sync.dma_start(w2_sb[:], w2_d[:])
            ident = ipool.tile([128, 128], w1_dt, tag="ident")
            masks.make_identity(nc, ident[:])

            # transposes per psum bank: bf16 bank holds 8 chunks, f32 bank 4
            per_bank = 8 if w1_dt == BF16 else 4
            bank_free = 128 * per_bank

            # blocks 0-5 in pairs (mm1 N=256); last two as singles so the
            # tail dependency chain (load->sub->transpose->mm1->mm2->store)
            # is short when the DMA stream runs dry of other work
            units = UNIT_LAYOUTS[layout]
            for unit in units:
                nu = len(unit)
                ots = []
                # Dt for the unit, block-major: dtt[p, D*par + 128*j + b]
                dtt = dtpool.tile([128, nu * D], w1_dt, tag="dtt")
                for par in range(nu):
                    i = unit[par]
                    # base loads straight into the output tile; the
                    # correction is accumulated in place later.
                    ot = opool.tile([128, D], F32, tag="ot")
                    nc.sync.dma_start(ot[:], base_d[128 * i : 128 * (i + 1), :])
                    st = spool.tile([128, D], F32, tag="st")
                    nc.sync.dma_start(st[:], src_d[128 * i : 128 * (i + 1), :])
                    ots.append(ot)

                    if mm1_dtype == "bf16":
                        db = dpool.tile([128, D], BF16, tag="db")
                        nc.vector.tensor_sub(db[:], st[:], ot[:])
                    else:
                        db = st  # subtract in place, keep f32
                        nc.vector.tensor_sub(db[:], st[:], ot[:])

                    use_dma_t = (layout == "all_dma") or (
                        layout == "tail_dma" and nu == 1
                    )
                    if use_dma_t:
                        # xbar transpose straight into dtt: with out viewed as
                        # [p, j, b], dtt[p, 128j+b] = db[b, 128j+p] — the same
                        # chunk layout the PE path produces.
                        d3 = dtt[:, D * par : D * (par + 1)].rearrange(
                            "p (j b) -> p j b", b=128
                        )
                        nc.sync.dma_start(d3, db[:], transpose=True)
                    else:
                        for g in range(NCH // per_bank):
                            ps = ptrpool.tile([128, bank_free], w1_dt, tag="ps")
                            for q in range(per_bank):
                                j = per_bank * g + q
                                nc.tensor.transpose(
                                    ps[:, 128 * q : 128 * (q + 1)],
                                    db[:, 128 * j : 128 * (j + 1)],
                                    ident[:],
                                )
                            nc.scalar.copy(
                                dtt[:, D * par + bank_free * g : D * par + bank_free * (g + 1)],
                                ps[:],
                            )

                # mm1: T^T for the unit, N=128*nu via 3D AP (par, b) over dtt
                dt3 = dtt[:].rearrange("p (par j b) -> p j par b", par=nu, b=128)
                pt = pTpool.tile([K, 128 * nu], F32, tag="pt")
                for j in range(NCH):
                    nc.tensor.matmul(
                        pt[:],
                        w1_sb[:, 128 * j : 128 * (j + 1)],
                        dt3[:, j],
                        start=(j == 0),
                        stop=(j == NCH - 1),
                    )
                ttt = ttpool.tile([K, 128 * nu], w2_dt, tag="ttt")
                nc.vector.tensor_copy(ttt[:], pt[:])

                for par in range(nu):
                    i = unit[par]
                    ot = ots[par]
                    for dj in range(D // 512):
                        p2t = p2pool.tile([128, 512], F32, tag="p2t")
                        lhs = ttt[:, 128 * par : 128 * (par + 1)]
                        rhs = w2_sb[:, 512 * dj : 512 * (dj + 1)]
                        nc.tensor.matmul(p2t[:], lhs, rhs, start=True, stop=True)
                        nc.vector.tensor_add(
                            ot[:, 512 * dj : 512 * (dj + 1)],
                            ot[:, 512 * dj : 512 * (dj + 1)],
                            p2t[:],
                        )
                    if nu == 1:
                        # stream the tail out in halves
                        half = D // 2
                        nc.sync.dma_start(
                            out_d[128 * i : 128 * (i + 1), :half], ot[:, :half]
                        )
                        nc.sync.dma_start(
                            out_d[128 * i : 128 * (i + 1), half:], ot[:, half:]
                        )
                    else:
                        nc.sync.dma_start(
                            out_d[128 * i : 128 * (i + 1), :], ot[:]
                        )

    nc.compile()
    return nc


_NC_CACHE = {}


def _get_nc(mm1_dtype, mm2_f32r, layout="tail_singles", deep_bufs=False):
    key = (mm1_dtype, mm2_f32r, layout, deep_bufs)
    if key not in _NC_CACHE:
        _NC_CACHE[key] = _build(mm1_dtype, mm2_f32r, layout, deep_bufs)
    return _NC_CACHE[key]


def make_in_maps(inputs, mm1_dtype="bf16"):
    base = np.ascontiguousarray(np.asarray(inputs["base"], dtype=np.float32))
    source = np.ascontiguousarray(np.asarray(inputs["source"], dtype=np.float32))
    subspaces = np.asarray(inputs["subspaces"])
    W = np.asarray(inputs["W"], dtype=np.float32)
    assert base.shape == (B_FULL, D) and source.shape == (B_FULL, D)

    sel = np.asarray(subspaces[0]).astype(np.int64)  # shared index set
    W_sel = np.ascontiguousarray(W[:, sel])          # [D, K] f32
    # chunk-major layout: w1[p, 128*j + k] = W_sel[128*j + p, k]
    w1 = np.ascontiguousarray(
        W_sel.reshape(NCH, 128, K).transpose(1, 0, 2).reshape(128, D)
    )
    if mm1_dtype == "bf16":
        w1 = w1.astype(ml_dtypes.bfloat16)
    w2 = np.ascontiguousarray(W_sel.T)               # [K, D] f32

    in_maps = []
    for c in range(N_CORES):
        in_maps.append(
            {
                "base": np.ascontiguousarray(base[c * BS : (c + 1) * BS]),
                "source": np.ascontiguousarray(source[c * BS : (c + 1) * BS]),
                "w1": w1,
                "w2": w2,
            }
        )
    return in_maps


def run(inputs, trace=False, mm1_dtype="bf16", mm2_f32r=False, layout="tail_singles", deep_bufs=False):
    nc = _get_nc(mm1_dtype, mm2_f32r, layout, deep_bufs)
    in_maps = make_in_maps(inputs, mm1_dtype)
    res = run_bass_kernel_spmd(nc, in_maps, list(range(N_CORES)), trace=trace)
    out = np.concatenate([r["out"] for r in res.results], axis=0)
    return out, res


def kernel(**inputs) -> np.ndarray:
    mm1_dtype = os.environ.get("LRI_MM1", "bf16")
    mm2_f32r = os.environ.get("LRI_MM2_F32R", "1") == "1"
    layout = os.environ.get("LRI_UNITS", "tail_singles")
    out, _ = run(inputs, trace=False, mm1_dtype=mm1_dtype, mm2_f32r=mm2_f32r, layout=layout)
    return out



# revision 2
# speedup vs baseline: 1.5610x; 1.5610x over previous
"""Trainium2 Bass kernel for nn_LowRankRotatedSpaceIntervention.

Reference computation (B=8192, D=4096, r=512, k=128):
    sel  = subspaces[0]                  # shared index set (fast path)
    diff = (source - base) @ W           # [B, r]
    out  = base + diff[:, sel] @ W[:, sel].T

Only the selected k=128 columns of W matter:
    out = base + ((source - base) @ W_sel) @ W_sel.T,  W_sel = W[:, sel]

The problem is HBM-bound (per-core DMA ceiling ~300 GB/s, PE needs only
~27us of matmul). So the kernel is organized purely around minimizing
HBM bytes and keeping every DMA large and contiguous:

  * base/source are packed on the host into a TRANSPOSED chunk-major
    bf16 layout, so the device needs no PE transposes at all: the
    contraction dim (d) is already on partitions.
  * all device I/O is 16-bit (bf16 in / bf16 out, optionally fp8 source);
    host converts back to f32. rel-err budget is 2e-2, bf16 rounding of
    base/out contributes ~2e-3.

Per-core layout (BS=1024 rows/core, 2 batch tiles of Tb=512):
  sT/bT dram [2*128, 32*512]:  [t*128+p, j*512+b] = x[t*512+b, j*128+p]
  device per batch tile t:
    diffT = sT - bT                       (DVE, in place, bf16)
    T^T[k,512]  = sum_j w1_j.T @ diffT_j  (32 bf16 matmuls, psum f32)
    tt = bf16(T^T)                        (scalar engine copy)
    per chunk j: corrT_j = w2_j.T @ tt    (matmul) ; outT_j = bT_j + corrT_j
    store outT groups of 8 chunks         ([128, 4096] bf16 stores)
"""

import numpy as np
import ml_dtypes

import concourse.bass as bass
import concourse.tile as tile
from concourse import bacc, mybir
from concourse.bass_utils import run_bass_kernel_spmd

N_CORES = 8
B_FULL = 8192
D = 4096
K = 128
BS = B_FULL // N_CORES   # 1024 rows per core
NT = 2                   # batch tiles per core
TB = BS // NT            # 512 batch rows per tile
NCH = D // 128           # 32 contraction / output chunks
GCH = 8                  # chunks per output store group
G = NCH // GCH           # 4 store groups per tile

F32 = mybir.dt.float32
BF16 = mybir.dt.bfloat16
FP8 = mybir.dt.float8e4


def _build(src_dtype="bf16"):
    nc = bacc.Bacc("TRN2", target_bir_lowering=False, debug=False)

    s_dt = BF16 if src_dtype == "bf16" else FP8
    sT_d = nc.dram_tensor("sT", [NT * 128, NCH * TB], s_dt, kind="ExternalInput").ap()
    bT_d = nc.dram_tensor("bT", [NT * 128, NCH * TB], BF16, kind="ExternalInput").ap()
    # w1: chunk-major W_sel: w1[p, 128*j + kk] = W_sel[128*j + p, kk]
    w1_d = nc.dram_tensor("w1", [128, D], BF16, kind="ExternalInput").ap()
    # w2: W_sel.T (k on partitions)
    w2_d = nc.dram_tensor("w2", [K, D], BF16, kind="ExternalInput").ap()
    out_d = nc.dram_tensor("out", [NT * G * 128, GCH * TB], BF16, kind="ExternalOutput").ap()

    in_place_sub = s_dt == BF16

    with tile.TileContext(nc) as tc:
        with (
            tc.tile_pool(name="wpool", bufs=1) as wpool,
            tc.tile_pool(name="spool", bufs=2) as spool,
            tc.tile_pool(name="bpool", bufs=2) as bpool,
            tc.tile_pool(name="dpool", bufs=2) as dpool,
            tc.tile_pool(name="ttpool", bufs=2) as ttpool,
            tc.tile_pool(name="opool", bufs=3) as opool,
            tc.tile_pool(name="pT", bufs=2, space="PSUM") as pTpool,
            tc.tile_pool(name="p2", bufs=4, space="PSUM") as p2pool,
        ):
            w1_sb = wpool.tile([128, D], BF16, tag="w1")
            nc.sync.dma_start(w1_sb[:], w1_d[:])
            w2_sb = wpool.tile([K, D], BF16, tag="w2")
            nc.sync.dma_start(w2_sb[:], w2_d[:])

            for t in range(NT):
                st = spool.tile([128, NCH * TB], s_dt, tag="st")
                bt = bpool.tile([128, NCH * TB], BF16, tag="bt")
                if in_place_sub:
                    dt = st
                else:
                    dt = dpool.tile([128, NCH * TB], BF16, tag="dt")
                rows = slice(128 * t, 128 * (t + 1))
                for g in range(G):
                    cols = slice(GCH * TB * g, GCH * TB * (g + 1))
                    nc.sync.dma_start(st[:, cols], sT_d[rows, cols])
                    nc.sync.dma_start(bt[:, cols], bT_d[rows, cols])
                    nc.vector.tensor_sub(dt[:, cols], st[:, cols], bt[:, cols])

                pt = pTpool.tile([K, TB], F32, tag="pt")
                for j in range(NCH):
                    nc.tensor.matmul(
                        pt[:],
                        w1_sb[:, 128 * j : 128 * (j + 1)],
                        dt[:, TB * j : TB * (j + 1)],
                        start=(j == 0),
                        stop=(j == NCH - 1),
                    )
                tt = ttpool.tile([K, TB], BF16, tag="tt")
                nc.scalar.copy(tt[:], pt[:])

                for g in range(G):
                    ot = opool.tile([128, GCH * TB], BF16, tag="ot")
                    for jj in range(GCH):
                        j = GCH * g + jj
                        p2 = p2pool.tile([128, TB], F32, tag="p2")
                        nc.tensor.matmul(
                            p2[:],
                            w2_sb[:, 128 * j : 128 * (j + 1)],
                            tt[:],
                            start=True,
                            stop=True,
                        )
                        nc.vector.tensor_add(
                            ot[:, TB * jj : TB * (jj + 1)],
                            bt[:, TB * j : TB * (j + 1)],
                            p2[:],
                        )
                    nc.sync.dma_start(
                        out_d[128 * (G * t + g) : 128 * (G * t + g + 1), :], ot[:]
                    )

    nc.compile()
    return nc


_NC_CACHE = {}


def _get_nc(src_dtype="bf16"):
    if src_dtype not in _NC_CACHE:
        _NC_CACHE[src_dtype] = _build(src_dtype)
    return _NC_CACHE[src_dtype]


def _pack_xT(x16):
    """[8192, 4096] -> [cores, NT*128, NCH*TB] transposed chunk-major."""
    v = x16.reshape(N_CORES, NT, TB, NCH, 128)
    return np.ascontiguousarray(v.transpose(0, 1, 4, 3, 2)).reshape(
        N_CORES, NT * 128, NCH * TB
    )


def make_in_maps(inputs, src_dtype="bf16"):
    base = np.asarray(inputs["base"], dtype=np.float32)
    source = np.asarray(inputs["source"], dtype=np.float32)
    subspaces = np.asarray(inputs["subspaces"])
    W = np.asarray(inputs["W"], dtype=np.float32)
    assert base.shape == (B_FULL, D) and source.shape == (B_FULL, D)

    sel = np.asarray(subspaces[0]).astype(np.int64)  # shared index set
    W_sel = np.ascontiguousarray(W[:, sel])          # [D, K] f32
    # chunk-major layout: w1[p, 128*j + kk] = W_sel[128*j + p, kk]
    w1 = np.ascontiguousarray(
        W_sel.reshape(NCH, 128, K).transpose(1, 0, 2).reshape(128, D)
    ).astype(ml_dtypes.bfloat16)
    w2 = np.ascontiguousarray(W_sel.T).astype(ml_dtypes.bfloat16)  # [K, D]

    s_np = ml_dtypes.bfloat16 if src_dtype == "bf16" else ml_dtypes.float8_e4m3
    sT = _pack_xT(source.astype(s_np))
    bT = _pack_xT(base.astype(ml_dtypes.bfloat16))

    in_maps = []
    for c in range(N_CORES):
        in_maps.append({"sT": sT[c], "bT": bT[c], "w1": w1, "w2": w2})
    return in_maps


def unpack_out(res_list):
    """Per-core [NT*G*128, GCH*TB] bf16 -> [8192, 4096] f32."""
    o = np.stack([r["out"] for r in res_list])
    # [c, t, g, p, jj, b] with d = (g*GCH + jj)*128 + p, batch = t*TB + b
    v = o.reshape(N_CORES, NT, G, 128, GCH, TB)
    out = v.transpose(0, 1, 5, 2, 4, 3).reshape(B_FULL, D)
    return np.ascontiguousarray(out).astype(np.float32)


def run(inputs, trace=False, src_dtype="bf16", **_ignored):
    nc = _get_nc(src_dtype)
    in_maps = make_in_maps(inputs, src_dtype)
    res = run_bass_kernel_spmd(nc, in_maps, list(range(N_CORES)), trace=trace)
    out = unpack_out(res.results)
    return out, res


def kernel(**inputs) -> np.ndarray:
    out, _ = run(inputs, trace=False)
    return out


# revision 4
# speedup vs baseline: 1.6930x; 1.0845x over previous
"""Trainium2 Bass kernel for nn_LowRankRotatedSpaceIntervention.

Reference computation (B=8192, D=4096, r=512, k=128):
    sel  = subspaces[0]                  # shared index set (fast path)
    diff = (source - base) @ W           # [B, r]
    out  = base + diff[:, sel] @ W[:, sel].T

Only the selected k=128 columns of W matter:
    out = base + ((source - base) @ W_sel) @ W_sel.T,  W_sel = W[:, sel]

The problem is HBM-bound (per-core DMA ceiling ~300 GB/s, PE needs only
~27us of matmul). So the kernel is organized purely around minimizing
HBM bytes and keeping every DMA large and contiguous:

  * base/source are packed on the host into a TRANSPOSED chunk-major
    bf16 layout, so the device needs no PE transposes at all: the
    contraction dim (d) is already on partitions.
  * all device I/O is 16-bit (bf16 in / bf16 out, optionally fp8 source);
    host converts back to f32. rel-err budget is 2e-2, bf16 rounding of
    base/out contributes ~2e-3.

Per-core layout (BS=1024 rows/core, 2 batch tiles of Tb=512):
  sT/bT dram [2*128, 32*512]:  [t*128+p, j*512+b] = x[t*512+b, j*128+p]
  device per batch tile t:
    diffT = sT - bT                       (DVE, in place, bf16)
    T^T[k,512]  = sum_j w1_j.T @ diffT_j  (32 bf16 matmuls, psum f32)
    tt = bf16(T^T)                        (scalar engine copy)
    per chunk j: corrT_j = w2_j.T @ tt    (matmul) ; outT_j = bT_j + corrT_j
    store outT groups of 8 chunks         ([128, 4096] bf16 stores)
"""

import numpy as np
import ml_dtypes

import concourse.bass as bass
import concourse.tile as tile
from concourse import bacc, mybir
from concourse.bass_utils import run_bass_kernel_spmd

N_CORES = 8
B_FULL = 8192
D = 4096
K = 128
BS = B_FULL // N_CORES   # 1024 rows per core
NT = 4                   # batch tiles per core
TB = BS // NT            # 256 batch rows per tile
NCH = D // 128           # 32 contraction / output chunks
GCH = 8                  # chunks per load/store group
G = NCH // GCH           # 4 groups per tile
PCH = 4                  # mm2 chunks drained per psum tile / DVE add

F32 = mybir.dt.float32
BF16 = mybir.dt.bfloat16
FP8 = mybir.dt.float8e4


def _build(src_dtype="bf16"):
    nc = bacc.Bacc("TRN2", target_bir_lowering=False, debug=False)

    s_dt = BF16 if src_dtype == "bf16" else FP8
    sT_d = nc.dram_tensor("sT", [NT * 128, NCH * TB], s_dt, kind="ExternalInput").ap()
    bT_d = nc.dram_tensor("bT", [NT * 128, NCH * TB], BF16, kind="ExternalInput").ap()
    # w1: chunk-major W_sel: w1[p, 128*j + kk] = W_sel[128*j + p, kk]
    w1_d = nc.dram_tensor("w1", [128, D], BF16, kind="ExternalInput").ap()
    # w2: W_sel.T (k on partitions)
    w2_d = nc.dram_tensor("w2", [K, D], BF16, kind="ExternalInput").ap()
    out_d = nc.dram_tensor("out", [NT * G * 128, GCH * TB], BF16, kind="ExternalOutput").ap()

    in_place_sub = s_dt == BF16

    with tile.TileContext(nc) as tc:
        with (
            tc.tile_pool(name="wpool", bufs=1) as wpool,
            tc.tile_pool(name="spool", bufs=NT) as spool,
            tc.tile_pool(name="bpool", bufs=NT) as bpool,
            tc.tile_pool(name="dpool", bufs=2) as dpool,
            tc.tile_pool(name="ttpool", bufs=2) as ttpool,
            tc.tile_pool(name="opool", bufs=4) as opool,
            tc.tile_pool(name="pT", bufs=2, space="PSUM") as pTpool,
            tc.tile_pool(name="p2", bufs=3, space="PSUM") as p2pool,
        ):
            w1_sb = wpool.tile([128, D], BF16, tag="w1")
            nc.sync.dma_start(w1_sb[:], w1_d[:])
            w2_sb = wpool.tile([K, D], BF16, tag="w2")

            for t in range(NT):
                st = spool.tile([128, NCH * TB], s_dt, tag="st")
                bt = bpool.tile([128, NCH * TB], BF16, tag="bt")
                if in_place_sub:
                    dt = st
                else:
                    dt = dpool.tile([128, NCH * TB], BF16, tag="dt")
                rows = slice(128 * t, 128 * (t + 1))
                for g in range(G):
                    cols = slice(GCH * TB * g, GCH * TB * (g + 1))
                    nc.sync.dma_start(st[:, cols], sT_d[rows, cols])
                    nc.sync.dma_start(bt[:, cols], bT_d[rows, cols])
                    nc.vector.tensor_sub(dt[:, cols], st[:, cols], bt[:, cols])
                if t == 0:
                    # w2 isn't needed until mm2(t0); don't let it delay
                    # the t0 input stream at the front of the DMA queues.
                    nc.sync.dma_start(w2_sb[:], w2_d[:])

                pt = pTpool.tile([K, TB], F32, tag="pt")
                for j in range(NCH):
                    nc.tensor.matmul(
                        pt[:],
                        w1_sb[:, 128 * j : 128 * (j + 1)],
                        dt[:, TB * j : TB * (j + 1)],
                        start=(j == 0),
                        stop=(j == NCH - 1),
                    )
                tt = ttpool.tile([K, TB], BF16, tag="tt")
                nc.scalar.copy(tt[:], pt[:])

                for g in range(G):
                    ot = opool.tile([128, GCH * TB], BF16, tag="ot")
                    for pg in range(GCH // PCH):
                        # PCH mm2 chunks into one 2-bank psum tile, drained
                        # by a single DVE add (fewer DVE instructions).
                        p2 = p2pool.tile([128, PCH * TB], F32, tag="p2")
                        for jj in range(PCH):
                            j = GCH * g + PCH * pg + jj
                            nc.tensor.matmul(
                                p2[:, TB * jj : TB * (jj + 1)],
                                w2_sb[:, 128 * j : 128 * (j + 1)],
                                tt[:],
                                start=True,
                                stop=True,
                            )
                        cols = slice(PCH * TB * pg, PCH * TB * (pg + 1))
                        j0 = GCH * g + PCH * pg
                        nc.vector.tensor_add(
                            ot[:, cols],
                            bt[:, TB * j0 : TB * (j0 + PCH)],
                            p2[:],
                        )
                    nc.sync.dma_start(
                        out_d[128 * (G * t + g) : 128 * (G * t + g + 1), :], ot[:]
                    )

    nc.compile()
    return nc


_NC_CACHE = {}


def _get_nc(src_dtype="bf16"):
    if src_dtype not in _NC_CACHE:
        _NC_CACHE[src_dtype] = _build(src_dtype)
    return _NC_CACHE[src_dtype]


def _pack_xT(x16):
    """[8192, 4096] -> [cores, NT*128, NCH*TB] transposed chunk-major."""
    v = x16.reshape(N_CORES, NT, TB, NCH, 128)
    return np.ascontiguousarray(v.transpose(0, 1, 4, 3, 2)).reshape(
        N_CORES, NT * 128, NCH * TB
    )


def make_in_maps(inputs, src_dtype="bf16"):
    base = np.asarray(inputs["base"], dtype=np.float32)
    source = np.asarray(inputs["source"], dtype=np.float32)
    subspaces = np.asarray(inputs["subspaces"])
    W = np.asarray(inputs["W"], dtype=np.float32)
    assert base.shape == (B_FULL, D) and source.shape == (B_FULL, D)

    sel = np.asarray(subspaces[0]).astype(np.int64)  # shared index set
    W_sel = np.ascontiguousarray(W[:, sel])          # [D, K] f32
    # chunk-major layout: w1[p, 128*j + kk] = W_sel[128*j + p, kk]
    w1 = np.ascontiguousarray(
        W_sel.reshape(NCH, 128, K).transpose(1, 0, 2).reshape(128, D)
    ).astype(ml_dtypes.bfloat16)
    w2 = np.ascontiguousarray(W_sel.T).astype(ml_dtypes.bfloat16)  # [K, D]

    s_np = ml_dtypes.bfloat16 if src_dtype == "bf16" else ml_dtypes.float8_e4m3
    sT = _pack_xT(source.astype(s_np))
    bT = _pack_xT(base.astype(ml_dtypes.bfloat16))

    in_maps = []
    for c in range(N_CORES):
        in_maps.append({"sT": sT[c], "bT": bT[c], "w1": w1, "w2": w2})
    return in_maps


def unpack_out(res_list):
    """Per-core [NT*G*128, GCH*TB] bf16 -> [8192, 4096] f32."""
    o = np.stack([r["out"] for r in res_list])
    # [c, t, g, p, jj, b] with d = (g*GCH + jj)*128 + p, batch = t*TB + b
    v = o.reshape(N_CORES, NT, G, 128, GCH, TB)
    out = v.transpose(0, 1, 5, 2, 4, 3).reshape(B_FULL, D)
    return np.ascontiguousarray(out).astype(np.float32)


def run(inputs, trace=False, src_dtype="bf16", **_ignored):
    nc = _get_nc(src_dtype)
    in_maps = make_in_maps(inputs, src_dtype)
    res = run_bass_kernel_spmd(nc, in_maps, list(range(N_CORES)), trace=trace)
    out = unpack_out(res.results)
    return out, res


def kernel(**inputs) -> np.ndarray:
    out, _ = run(inputs, trace=False)
    return out


# revision 5
# speedup vs baseline: 1.8657x; 1.1020x over previous
"""Trainium2 Bass kernel for nn_LowRankRotatedSpaceIntervention.

Reference computation (B=8192, D=4096, r=512, k=128):
    sel  = subspaces[0]                  # shared index set (fast path)
    diff = (source - base) @ W           # [B, r]
    out  = base + diff[:, sel] @ W[:, sel].T

Only the selected k=128 columns of W matter:
    out = base + ((source - base) @ W_sel) @ W_sel.T,  W_sel = W[:, sel]

The problem is HBM-bound (per-core DMA ceiling ~300 GB/s, PE needs only
~27us of matmul). So the kernel is organized purely around minimizing
HBM bytes and keeping every DMA large and contiguous:

  * base/source are packed on the host into a TRANSPOSED chunk-major
    bf16 layout, so the device needs no PE transposes at all: the
    contraction dim (d) is already on partitions.
  * all device I/O is 16-bit (bf16 in / bf16 out, optionally fp8 source);
    host converts back to f32. rel-err budget is 2e-2, bf16 rounding of
    base/out contributes ~2e-3.

Per-core layout (BS=1024 rows/core, 2 batch tiles of Tb=512):
  sT/bT dram [2*128, 32*512]:  [t*128+p, j*512+b] = x[t*512+b, j*128+p]
  device per batch tile t:
    diffT = sT - bT                       (DVE, in place, bf16)
    T^T[k,512]  = sum_j w1_j.T @ diffT_j  (32 bf16 matmuls, psum f32)
    tt = bf16(T^T)                        (scalar engine copy)
    per chunk j: corrT_j = w2_j.T @ tt    (matmul) ; outT_j = bT_j + corrT_j
    store outT groups of 8 chunks         ([128, 4096] bf16 stores)
"""

import numpy as np
import ml_dtypes

import concourse.bass as bass
import concourse.tile as tile
from concourse import bacc, mybir
from concourse.bass_utils import run_bass_kernel_spmd

N_CORES = 8
B_FULL = 8192
D = 4096
K = 128
BS = B_FULL // N_CORES   # 1024 rows per core
NT = 4                   # batch tiles per core
TB = BS // NT            # 256 batch rows per tile
NCH = D // 128           # 32 contraction / output chunks
GCH = 8                  # chunks per load/store group
G = NCH // GCH           # 4 groups per tile
PCH = 4                  # mm2 chunks drained per psum tile / DVE add

F32 = mybir.dt.float32
BF16 = mybir.dt.bfloat16
FP8 = mybir.dt.float8e4


def _build(src_dtype="bf16"):
    nc = bacc.Bacc("TRN2", target_bir_lowering=False, debug=False)

    s_dt = BF16 if src_dtype == "bf16" else FP8
    sT_d = nc.dram_tensor("sT", [NT * 128, NCH * TB], s_dt, kind="ExternalInput").ap()
    bT_d = nc.dram_tensor("bT", [NT * 128, NCH * TB], BF16, kind="ExternalInput").ap()
    # w1: chunk-major W_sel: w1[p, 128*j + kk] = W_sel[128*j + p, kk]
    w1_d = nc.dram_tensor("w1", [128, D], BF16, kind="ExternalInput").ap()
    # w2: W_sel.T (k on partitions)
    w2_d = nc.dram_tensor("w2", [K, D], BF16, kind="ExternalInput").ap()
    out_d = nc.dram_tensor("out", [NT * G * 128, GCH * TB], BF16, kind="ExternalOutput").ap()

    in_place_sub = s_dt == BF16

    with tile.TileContext(nc) as tc:
        with (
            tc.tile_pool(name="wpool", bufs=1) as wpool,
            tc.tile_pool(name="spool", bufs=NT) as spool,
            tc.tile_pool(name="bpool", bufs=NT) as bpool,
            tc.tile_pool(name="dpool", bufs=2) as dpool,
            tc.tile_pool(name="ttpool", bufs=2) as ttpool,
            tc.tile_pool(name="opool", bufs=4) as opool,
            tc.tile_pool(name="pT", bufs=2, space="PSUM") as pTpool,
            tc.tile_pool(name="p2", bufs=3, space="PSUM") as p2pool,
        ):
            w1_sb = wpool.tile([128, D], BF16, tag="w1")
            w2_sb = wpool.tile([K, D], BF16, tag="w2")
            # split w1 so mm1(t0) group g only waits on its slice
            for g in range(G):
                cols = slice(GCH * 128 * g, GCH * 128 * (g + 1))
                nc.sync.dma_start(w1_sb[:, cols], w1_d[:, cols])

            def emit_load(t):
                """Loads + subs for tile t (DVE work ahead of older adds)."""
                st = spool.tile([128, NCH * TB], s_dt, tag="st")
                bt = bpool.tile([128, NCH * TB], BF16, tag="bt")
                if in_place_sub:
                    dt = st
                else:
                    dt = dpool.tile([128, NCH * TB], BF16, tag="dt")
                rows = slice(128 * t, 128 * (t + 1))
                for g in range(G):
                    cols = slice(GCH * TB * g, GCH * TB * (g + 1))
                    nc.sync.dma_start(st[:, cols], sT_d[rows, cols])
                    nc.sync.dma_start(bt[:, cols], bT_d[rows, cols])
                    if in_place_sub:
                        nc.vector.tensor_sub(dt[:, cols], st[:, cols], bt[:, cols])
                    else:
                        # fp8 -> bf16 on the (idle) activation engine keeps
                        # the DVE sub in its 2x 16-bit mode.
                        nc.scalar.copy(dt[:, cols], st[:, cols])
                        nc.vector.tensor_sub(dt[:, cols], dt[:, cols], bt[:, cols])
                return bt, dt

            def emit_compute(t, bt, dt):
                pt = pTpool.tile([K, TB], F32, tag="pt")
                for j in range(NCH):
                    nc.tensor.matmul(
                        pt[:],
                        w1_sb[:, 128 * j : 128 * (j + 1)],
                        dt[:, TB * j : TB * (j + 1)],
                        start=(j == 0),
                        stop=(j == NCH - 1),
                    )
                tt = ttpool.tile([K, TB], BF16, tag="tt")
                nc.scalar.copy(tt[:], pt[:])

                for g in range(G):
                    ot = opool.tile([128, GCH * TB], BF16, tag="ot")
                    for pg in range(GCH // PCH):
                        # PCH mm2 chunks into one 2-bank psum tile, drained
                        # by a single DVE add (fewer DVE instructions).
                        p2 = p2pool.tile([128, PCH * TB], F32, tag="p2")
                        for jj in range(PCH):
                            j = GCH * g + PCH * pg + jj
                            nc.tensor.matmul(
                                p2[:, TB * jj : TB * (jj + 1)],
                                w2_sb[:, 128 * j : 128 * (j + 1)],
                                tt[:],
                                start=True,
                                stop=True,
                            )
                        cols = slice(PCH * TB * pg, PCH * TB * (pg + 1))
                        j0 = GCH * g + PCH * pg
                        nc.vector.tensor_add(
                            ot[:, cols],
                            bt[:, TB * j0 : TB * (j0 + PCH)],
                            p2[:],
                        )
                    nc.sync.dma_start(
                        out_d[128 * (G * t + g) : 128 * (G * t + g + 1), :], ot[:]
                    )

            # Software-pipelined emission: tile t+1's loads+subs are emitted
            # BEFORE tile t's mm/add phase, so the in-order DVE runs
            # subs(t+1) ahead of adds(t) and mm1(t+1) is never gated on the
            # psum-draining adds of the previous tile.
            tiles = {}
            tiles[0] = emit_load(0)
            nc.sync.dma_start(w2_sb[:], w2_d[:])
            for t in range(1, NT):
                tiles[t] = emit_load(t)
                emit_compute(t - 1, *tiles[t - 1])
            emit_compute(NT - 1, *tiles[NT - 1])

    nc.compile()
    return nc


_NC_CACHE = {}


def _get_nc(src_dtype="bf16"):
    if src_dtype not in _NC_CACHE:
        _NC_CACHE[src_dtype] = _build(src_dtype)
    return _NC_CACHE[src_dtype]


def _pack_xT(x16):
    """[8192, 4096] -> [cores, NT*128, NCH*TB] transposed chunk-major."""
    v = x16.reshape(N_CORES, NT, TB, NCH, 128)
    return np.ascontiguousarray(v.transpose(0, 1, 4, 3, 2)).reshape(
        N_CORES, NT * 128, NCH * TB
    )


def make_in_maps(inputs, src_dtype="bf16"):
    base = np.asarray(inputs["base"], dtype=np.float32)
    source = np.asarray(inputs["source"], dtype=np.float32)
    subspaces = np.asarray(inputs["subspaces"])
    W = np.asarray(inputs["W"], dtype=np.float32)
    assert base.shape == (B_FULL, D) and source.shape == (B_FULL, D)

    sel = np.asarray(subspaces[0]).astype(np.int64)  # shared index set
    W_sel = np.ascontiguousarray(W[:, sel])          # [D, K] f32
    # chunk-major layout: w1[p, 128*j + kk] = W_sel[128*j + p, kk]
    w1 = np.ascontiguousarray(
        W_sel.reshape(NCH, 128, K).transpose(1, 0, 2).reshape(128, D)
    ).astype(ml_dtypes.bfloat16)
    w2 = np.ascontiguousarray(W_sel.T).astype(ml_dtypes.bfloat16)  # [K, D]

    s_np = ml_dtypes.bfloat16 if src_dtype == "bf16" else ml_dtypes.float8_e4m3
    sT = _pack_xT(source.astype(s_np))
    bT = _pack_xT(base.astype(ml_dtypes.bfloat16))

    in_maps = []
    for c in range(N_CORES):
        in_maps.append({"sT": sT[c], "bT": bT[c], "w1": w1, "w2": w2})
    return in_maps


def unpack_out(res_list):
    """Per-core [NT*G*128, GCH*TB] bf16 -> [8192, 4096] f32."""
    o = np.stack([r["out"] for r in res_list])
    # [c, t, g, p, jj, b] with d = (g*GCH + jj)*128 + p, batch = t*TB + b
    v = o.reshape(N_CORES, NT, G, 128, GCH, TB)
    out = v.transpose(0, 1, 5, 2, 4, 3).reshape(B_FULL, D)
    return np.ascontiguousarray(out).astype(np.float32)


def run(inputs, trace=False, src_dtype="bf16", **_ignored):
    nc = _get_nc(src_dtype)
    in_maps = make_in_maps(inputs, src_dtype)
    res = run_bass_kernel_spmd(nc, in_maps, list(range(N_CORES)), trace=trace)
    out = unpack_out(res.results)
    return out, res


def kernel(**inputs) -> np.ndarray:
    out, _ = run(inputs, trace=False)
    return out
